# revision 15
# baseline (speedup 1.0000x reference)
"""BjorckLinear Trainium2 kernel: y = x @ bjorck(kernel/1024, beta=0.5, iters=20) + bias.

Self-contained: 8-core SPMD, data-parallel over rows of x, Bjorck iteration
replicated per core in fp32r (full-rate PE matmuls, ~tf32 precision).

Layouts (host prepares, all free for HW time):
  - x (4,8192,1024) -> flatten (32768,1024) -> per-core shard (4096,1024)
    -> transposed xT (1024,4096) contiguous.
  - w0 = kernel/1024 replicated (1024,1024).
  - bias packed as (128,8): bias_pk[p, j] = bias[j*128 + p].
  - output: per-core yT (1024,4096); host transposes/concats back.

Device program per core:
  w = w0
  repeat 20: A = w^T w ; wn = 1.5 w - 0.5 (v^T A) with v = w^T ; v refreshed
             by PE transposes of wn.
  yT[n,:] = sum_k w[k,n-slice]^T xT[k,:] + bias  (w stationary, LDW amortized)
"""
import os
import sys
import numpy as np

_TRN_REPO = "/opt/trn_rl_repo"
if _TRN_REPO not in sys.path and os.path.isdir(_TRN_REPO):
    sys.path.insert(0, _TRN_REPO)

import concourse.bacc as bacc
import concourse.mybir as mybir
import concourse.tile as tile
from concourse import masks
from concourse.bass_utils import run_bass_kernel_spmd

N_CORES = 8
ITERS = int(os.environ.get("BJORCK_ITERS", "20"))
D = 1024                   # feature dim
ROWS_PER_CORE = 4096       # 4*8192/8
KT = D // 128              # 8 k-tiles
MH = ROWS_PER_CORE // 2    # 2048, m-half for xT streaming
f32 = mybir.dt.float32
f32r = mybir.dt.float32r


def _build():
    nc = bacc.Bacc(None, target_bir_lowering=False, debug=False)

    w0_d = nc.declare_dram_parameter("w0", [D, D], f32, isOutput=False)
    w0T_d = nc.declare_dram_parameter("w0T", [D, D], f32, isOutput=False)
    xT_d = nc.declare_dram_parameter("xT", [D, ROWS_PER_CORE], f32, isOutput=False)
    bias_d = nc.declare_dram_parameter("bias_pk", [128, KT], f32, isOutput=False)
    yT_d = nc.declare_dram_parameter("yT", [D, ROWS_PER_CORE], f32, isOutput=True)

    with tile.TileContext(nc) as tc:
        with (
            tc.tile_pool(name="persist", bufs=1) as persist,
            tc.tile_pool(name="wpool", bufs=2) as wpool,
        ):
            ident = persist.tile([128, 128], f32, name="ident")
            masks.make_identity(nc, ident[:])
            bias_sb = persist.tile([128, KT], f32, name="bias_sb")
            nc.sync.dma_start(bias_sb[:], bias_d[:])

            # --- load w0 (k-tiles) and build v0 = w0^T via PE transpose
            w = [wpool.tile([128, D], f32r, tag=f"w{k}", name=f"w_{k}") for k in range(KT)]
            for k in range(KT):
                nc.sync.dma_start(w[k][:], w0_d[k * 128:(k + 1) * 128, :].bitcast(f32r))

            # x half-0 prefetch pool lives through Bjorck so its DMAs overlap
            xpool_cm = tc.tile_pool(name="xpool", bufs=1)
            xpool = xpool_cm.__enter__()
            xh0 = [xpool.tile([128, MH], f32r, tag=f"x{k}", name=f"x0_{k}")
                   for k in range(KT)]
            for k in range(KT):
                nc.sync.dma_start(xh0[k][:], xT_d[k * 128:(k + 1) * 128, 0:MH]
                                  .bitcast(f32r))

            # --- Bjorck iterations ---
            with (
                tc.tile_pool(name="vpool", bufs=1) as vpool,
                tc.tile_pool(name="apool", bufs=1) as apool,
                tc.tile_pool(name="ps_mm", bufs=4, space="PSUM") as ps_mm,
                tc.tile_pool(name="ps_tr", bufs=4, space="PSUM") as ps_tr,
            ):
                # v0 = w0^T provided by the host
                v = [vpool.tile([128, D], f32r, tag=f"v{k}", name=f"v0_{k}")
                     for k in range(KT)]
                for k in range(KT):
                    nc.sync.dma_start(
                        v[k][:], w0T_d[k * 128:(k + 1) * 128, :].bitcast(f32r))

                for it in range(ITERS):
                    # A = w^T @ w   (A[m-tile] rows = 128 cols of w).
                    # A is symmetric: skip [128x512] units fully below the
                    # diagonal (m*128 >= (nb+1)*512) and mirror them from the
                    # transposed upper blocks instead.
                    A = [apool.tile([128, D], f32r, tag=f"A{m}", name=f"A{it}_{m}")
                         for m in range(KT)]
                    for m in range(KT):
                        for nb in range(2):
                            if m * 128 >= (nb + 1) * 512:
                                continue
                            acc = ps_mm.tile([128, 512], f32, tag="pmm")
                            for k in range(KT):
                                nc.tensor.matmul(
                                    acc[:],
                                    w[k][:, m * 128:(m + 1) * 128],
                                    w[k][:, nb * 512:(nb + 1) * 512],
                                    start=(k == 0), stop=(k == KT - 1))
                            nc.vector.tensor_copy(
                                A[m][:, nb * 512:(nb + 1) * 512], acc[:].bitcast(f32r))
                    # mirror: A[i-tile][:, j*128..] = transpose(A[j-tile][:, i*128..])
                    for i in range(4, KT):
                        for j in range(0, 4):
                            pt = ps_tr.tile([128, 128], f32, tag="ptr")
                            nc.tensor.transpose(
                                pt[:], A[j][:, i * 128:(i + 1) * 128].bitcast(f32),
                                ident[:])
                            nc.vector.tensor_copy(
                                A[i][:, j * 128:(j + 1) * 128], pt[:].bitcast(f32r))

                    # wn = 1.5 w - 0.5 * (v^T A)  ( (v^T A)[m-tile] = w@A rows )
                    wn = [wpool.tile([128, D], f32r, tag=f"w{m}", name=f"w{it+1}_{m}")
                          for m in range(KT)]
                    for m in range(KT):
                        for nb in range(2):
                            acc = ps_mm.tile([128, 512], f32, tag="pmm")
                            for k in range(KT):
                                nc.tensor.matmul(
                                    acc[:],
                                    v[k][:, m * 128:(m + 1) * 128],
                                    A[k][:, nb * 512:(nb + 1) * 512],
                                    start=(k == 0), stop=(k == KT - 1))
                            sl = slice(nb * 512, (nb + 1) * 512)
                            nc.vector.tensor_scalar_mul(
                                wn[m][:, sl], acc[:].bitcast(f32r), -0.5)
                            nc.vector.scalar_tensor_tensor(
                                out=wn[m][:, sl], in0=w[m][:, sl], scalar=1.5,
                                in1=wn[m][:, sl],
                                op0=mybir.AluOpType.mult, op1=mybir.AluOpType.add)

                    # v <- wn^T (skip on last iteration)
                    if it < ITERS - 1:
                        vn = [vpool.tile([128, D], f32r, tag=f"v{c}", name=f"v{it+1}_{c}")
                              for c in range(KT)]
                        for c in range(KT):
                            for m in range(KT):
                                pt = ps_tr.tile([128, 128], f32, tag="ptr")
                                nc.tensor.transpose(
                                    pt[:], wn[m][:, c * 128:(c + 1) * 128].bitcast(f32),
                                    ident[:])
                                nc.vector.tensor_copy(
                                    vn[c][:, m * 128:(m + 1) * 128],
                                    pt[:].bitcast(f32r))
                        v = vn
                    w = wn

            # --- main matmul: yT[n-tile] = sum_k w[k][:, n]^T @ xT[k] + bias ---
            with (
                tc.tile_pool(name="ypool", bufs=2) as ypool,
                tc.tile_pool(name="ps_y", bufs=2, space="PSUM") as ps_y,
            ):
                for half in range(2):
                    if half == 0:
                        xh = xh0
                    else:
                        xh = [xpool.tile([128, MH], f32r, tag=f"x{k}",
                                         name=f"x{half}_{k}") for k in range(KT)]
                        for k in range(KT):
                            nc.sync.dma_start(
                                xh[k][:],
                                xT_d[k * 128:(k + 1) * 128,
                                     half * MH:(half + 1) * MH].bitcast(f32r))
                    for n in range(KT):
                        banks = [ps_y.tile([128, 512], f32, tag=f"b{mb}",
                                           name=f"bank{half}_{n}_{mb}")
                                 for mb in range(MH // 512)]
                        for k in range(KT):
                            for mb in range(MH // 512):
                                nc.tensor.matmul(
                                    banks[mb][:],
                                    w[k][:, n * 128:(n + 1) * 128],
                                    xh[k][:, mb * 512:(mb + 1) * 512],
                                    start=(k == 0), stop=(k == KT - 1))
                        yt = ypool.tile([128, MH], f32, tag="yt", name=f"y{half}_{n}")
                        for mb in range(MH // 512):
                            nc.scalar.activation(
                                yt[:, mb * 512:(mb + 1) * 512], banks[mb][:],
                                mybir.ActivationFunctionType.Identity,
                                bias=bias_sb[:, n:n + 1], scale=1.0)
                        nc.sync.dma_start(
                            yT_d[n * 128:(n + 1) * 128, half * MH:(half + 1) * MH],
                            yt[:])
            xpool_cm.__exit__(None, None, None)
    nc.compile()
    return nc


_NC_CACHE = None


def _get_nc():
    global _NC_CACHE
    if _NC_CACHE is None:
        _NC_CACHE = _build()
    return _NC_CACHE


def run(x, kernel, bias, trace=False):
    """Returns (y, exec_time_ns)."""
    x = np.asarray(x, dtype=np.float32)
    kernel = np.asarray(kernel, dtype=np.float32)
    bias = np.asarray(bias, dtype=np.float32)

    w0 = (kernel / np.float32(np.sqrt(float(kernel.shape[0] * kernel.shape[1])))
          ).astype(np.float32)
    bias_pk = np.ascontiguousarray(bias.reshape(KT, 128).T)
    xf = x.reshape(-1, D)
    shards = [np.ascontiguousarray(xf[i * ROWS_PER_CORE:(i + 1) * ROWS_PER_CORE].T)
              for i in range(N_CORES)]
    w0T = np.ascontiguousarray(w0.T)
    in_maps = [{"w0": w0, "w0T": w0T, "xT": shards[i], "bias_pk": bias_pk}
               for i in range(N_CORES)]

    nc = _get_nc()
    r = run_bass_kernel_spmd(nc, in_maps, list(range(N_CORES)), trace=trace)
    y = np.concatenate([r.results[c]["yT"].T for c in range(N_CORES)], axis=0)
    return y.reshape(x.shape).astype(np.float32), r.exec_time_ns


def kernel(**inputs):
    y, _ = run(inputs["x"], inputs["kernel"], inputs["bias"])
    return y


# revision 17
# speedup vs baseline: 1.0076x; 1.0076x over previous
"""BjorckLinear Trainium2 kernel: y = x @ bjorck(kernel/1024, beta=0.5, iters=20) + bias.

Self-contained: 8-core SPMD, data-parallel over rows of x, Bjorck iteration
replicated per core in fp32r (full-rate PE matmuls, ~tf32 precision).

Layouts (host prepares, all free for HW time):
  - x (4,8192,1024) -> flatten (32768,1024) -> per-core shard (4096,1024)
    -> transposed xT (1024,4096) contiguous.
  - w0 = kernel/1024 replicated (1024,1024).
  - bias packed as (128,8): bias_pk[p, j] = bias[j*128 + p].
  - output: per-core yT (1024,4096); host transposes/concats back.

Device program per core:
  w = w0
  repeat 20: A = w^T w ; wn = 1.5 w - 0.5 (v^T A) with v = w^T ; v refreshed
             by PE transposes of wn.
  yT[n,:] = sum_k w[k,n-slice]^T xT[k,:] + bias  (w stationary, LDW amortized)
"""
import os
import sys
import numpy as np

_TRN_REPO = "/opt/trn_rl_repo"
if _TRN_REPO not in sys.path and os.path.isdir(_TRN_REPO):
    sys.path.insert(0, _TRN_REPO)

import concourse.bacc as bacc
import concourse.mybir as mybir
import concourse.tile as tile
from concourse import masks
from concourse.bass_utils import run_bass_kernel_spmd

def _ensure_ntff_hook():
    """Best-effort install of the antenv.axon_hooks module that
    run_bass_kernel_spmd(trace=True) needs under axon. Safe no-op on failure."""
    import types
    if "antenv.axon_hooks" not in sys.modules:
        mod = types.ModuleType("antenv.axon_hooks")
        hook = [None]
        mod.set_axon_ntff_profile_hook = lambda h: hook.__setitem__(0, h)
        mod.get_axon_ntff_profile_hook = lambda: hook[0]
        sys.modules["antenv.axon_hooks"] = mod
        try:
            import antenv
            antenv.axon_hooks = mod
        except ImportError:
            pass
    mod = sys.modules["antenv.axon_hooks"]
    if mod.get_axon_ntff_profile_hook() is None:
        try:
            from trn_agent_boot.trn_boot import _ntff_profile_via_ctypes
            mod.set_axon_ntff_profile_hook(
                _ntff_profile_via_ctypes("/opt/axon/libaxon_pjrt.so"))
        except Exception:
            pass


N_CORES = 8
ITERS = int(os.environ.get("BJORCK_ITERS", "20"))
D = 1024                   # feature dim
ROWS_PER_CORE = 4096       # 4*8192/8
KT = D // 128              # 8 k-tiles
MH = ROWS_PER_CORE // 2    # 2048, m-half for xT streaming
f32 = mybir.dt.float32
f32r = mybir.dt.float32r


def _build():
    nc = bacc.Bacc(None, target_bir_lowering=False, debug=False)

    w0_d = nc.declare_dram_parameter("w0", [D, D], f32, isOutput=False)
    w0T_d = nc.declare_dram_parameter("w0T", [D, D], f32, isOutput=False)
    xT_d = nc.declare_dram_parameter("xT", [D, ROWS_PER_CORE], f32, isOutput=False)
    bias_d = nc.declare_dram_parameter("bias_pk", [128, KT], f32, isOutput=False)
    yT_d = nc.declare_dram_parameter("yT", [D, ROWS_PER_CORE], f32, isOutput=True)

    with tile.TileContext(nc) as tc:
        with (
            tc.tile_pool(name="persist", bufs=1) as persist,
            tc.tile_pool(name="wpool", bufs=2) as wpool,
        ):
            ident = persist.tile([128, 128], f32, name="ident")
            masks.make_identity(nc, ident[:])
            bias_sb = persist.tile([128, KT], f32, name="bias_sb")
            nc.sync.dma_start(bias_sb[:], bias_d[:])

            # --- load w0 (k-tiles) and build v0 = w0^T via PE transpose
            w = [wpool.tile([128, D], f32r, tag=f"w{k}", name=f"w_{k}") for k in range(KT)]
            for k in range(KT):
                nc.sync.dma_start(w[k][:], w0_d[k * 128:(k + 1) * 128, :].bitcast(f32r))

            # x half-0 prefetch pool lives through Bjorck so its DMAs overlap
            xpool_cm = tc.tile_pool(name="xpool", bufs=1)
            xpool = xpool_cm.__enter__()
            xh0 = [xpool.tile([128, MH], f32r, tag=f"x{k}", name=f"x0_{k}")
                   for k in range(KT)]
            for k in range(KT):
                nc.sync.dma_start(xh0[k][:], xT_d[k * 128:(k + 1) * 128, 0:MH]
                                  .bitcast(f32r))

            # --- Bjorck iterations ---
            with (
                tc.tile_pool(name="vpool", bufs=1) as vpool,
                tc.tile_pool(name="apool", bufs=1) as apool,
                tc.tile_pool(name="ps_mm", bufs=4, space="PSUM") as ps_mm,
                tc.tile_pool(name="ps_tr", bufs=4, space="PSUM") as ps_tr,
            ):
                # v0 = w0^T provided by the host
                v = [vpool.tile([128, D], f32r, tag=f"v{k}", name=f"v0_{k}")
                     for k in range(KT)]
                for k in range(KT):
                    nc.sync.dma_start(
                        v[k][:], w0T_d[k * 128:(k + 1) * 128, :].bitcast(f32r))

                for it in range(ITERS):
                    # A = w^T @ w   (A[m-tile] rows = 128 cols of w).
                    # A is symmetric: skip [128x512] units fully below the
                    # diagonal (m*128 >= (nb+1)*512) and mirror them from the
                    # transposed upper blocks instead.
                    A = [apool.tile([128, D], f32r, tag=f"A{m}", name=f"A{it}_{m}")
                         for m in range(KT)]
                    for m in range(KT):
                        for nb in range(2):
                            if m * 128 >= (nb + 1) * 512:
                                continue
                            acc = ps_mm.tile([128, 512], f32, tag="pmm")
                            for k in range(KT):
                                nc.tensor.matmul(
                                    acc[:],
                                    w[k][:, m * 128:(m + 1) * 128],
                                    w[k][:, nb * 512:(nb + 1) * 512],
                                    start=(k == 0), stop=(k == KT - 1))
                            nc.vector.tensor_copy(
                                A[m][:, nb * 512:(nb + 1) * 512], acc[:].bitcast(f32r))
                    # mirror: A[i-tile][:, j*128..] = transpose(A[j-tile][:, i*128..])
                    for i in range(4, KT):
                        for j in range(0, 4):
                            pt = ps_tr.tile([128, 128], f32, tag="ptr")
                            nc.tensor.transpose(
                                pt[:], A[j][:, i * 128:(i + 1) * 128].bitcast(f32),
                                ident[:])
                            nc.vector.tensor_copy(
                                A[i][:, j * 128:(j + 1) * 128], pt[:].bitcast(f32r))

                    # wn = 1.5 w - 0.5 * (v^T A)  ( (v^T A)[m-tile] = w@A rows )
                    wn = [wpool.tile([128, D], f32r, tag=f"w{m}", name=f"w{it+1}_{m}")
                          for m in range(KT)]
                    for m in range(KT):
                        for nb in range(2):
                            acc = ps_mm.tile([128, 512], f32, tag="pmm")
                            for k in range(KT):
                                nc.tensor.matmul(
                                    acc[:],
                                    v[k][:, m * 128:(m + 1) * 128],
                                    A[k][:, nb * 512:(nb + 1) * 512],
                                    start=(k == 0), stop=(k == KT - 1))
                            sl = slice(nb * 512, (nb + 1) * 512)
                            nc.vector.tensor_scalar_mul(
                                wn[m][:, sl], acc[:].bitcast(f32r), -0.5)
                            nc.vector.scalar_tensor_tensor(
                                out=wn[m][:, sl], in0=w[m][:, sl], scalar=1.5,
                                in1=wn[m][:, sl],
                                op0=mybir.AluOpType.mult, op1=mybir.AluOpType.add)

                    # v <- wn^T (skip on last iteration)
                    if it < ITERS - 1:
                        vn = [vpool.tile([128, D], f32r, tag=f"v{c}", name=f"v{it+1}_{c}")
                              for c in range(KT)]
                        for c in range(KT):
                            for m in range(KT):
                                pt = ps_tr.tile([128, 128], f32, tag="ptr")
                                nc.tensor.transpose(
                                    pt[:], wn[m][:, c * 128:(c + 1) * 128].bitcast(f32),
                                    ident[:])
                                nc.vector.tensor_copy(
                                    vn[c][:, m * 128:(m + 1) * 128],
                                    pt[:].bitcast(f32r))
                        v = vn
                    w = wn

            # --- main matmul: yT[n-tile] = sum_k w[k][:, n]^T @ xT[k] + bias ---
            with (
                tc.tile_pool(name="ypool", bufs=2) as ypool,
                tc.tile_pool(name="ps_y", bufs=2, space="PSUM") as ps_y,
            ):
                for half in range(2):
                    if half == 0:
                        xh = xh0
                    else:
                        xh = [xpool.tile([128, MH], f32r, tag=f"x{k}",
                                         name=f"x{half}_{k}") for k in range(KT)]
                        for k in range(KT):
                            nc.sync.dma_start(
                                xh[k][:],
                                xT_d[k * 128:(k + 1) * 128,
                                     half * MH:(half + 1) * MH].bitcast(f32r))
                    for n in range(KT):
                        banks = [ps_y.tile([128, 512], f32, tag=f"b{mb}",
                                           name=f"bank{half}_{n}_{mb}")
                                 for mb in range(MH // 512)]
                        for k in range(KT):
                            for mb in range(MH // 512):
                                nc.tensor.matmul(
                                    banks[mb][:],
                                    w[k][:, n * 128:(n + 1) * 128],
                                    xh[k][:, mb * 512:(mb + 1) * 512],
                                    start=(k == 0), stop=(k == KT - 1))
                        yt = ypool.tile([128, MH], f32, tag="yt", name=f"y{half}_{n}")
                        for mb in range(MH // 512):
                            nc.scalar.activation(
                                yt[:, mb * 512:(mb + 1) * 512], banks[mb][:],
                                mybir.ActivationFunctionType.Identity,
                                bias=bias_sb[:, n:n + 1], scale=1.0)
                        nc.sync.dma_start(
                            yT_d[n * 128:(n + 1) * 128, half * MH:(half + 1) * MH],
                            yt[:])
            xpool_cm.__exit__(None, None, None)
    nc.compile()
    return nc


_NC_CACHE = None


def _get_nc():
    global _NC_CACHE
    if _NC_CACHE is None:
        _NC_CACHE = _build()
    return _NC_CACHE


def run(x, kernel, bias, trace=False):
    """Returns (y, exec_time_ns)."""
    x = np.asarray(x, dtype=np.float32)
    kernel = np.asarray(kernel, dtype=np.float32)
    bias = np.asarray(bias, dtype=np.float32)

    w0 = (kernel / np.float32(np.sqrt(float(kernel.shape[0] * kernel.shape[1])))
          ).astype(np.float32)
    bias_pk = np.ascontiguousarray(bias.reshape(KT, 128).T)
    xf = x.reshape(-1, D)
    shards = [np.ascontiguousarray(xf[i * ROWS_PER_CORE:(i + 1) * ROWS_PER_CORE].T)
              for i in range(N_CORES)]
    w0T = np.ascontiguousarray(w0.T)
    in_maps = [{"w0": w0, "w0T": w0T, "xT": shards[i], "bias_pk": bias_pk}
               for i in range(N_CORES)]

    nc = _get_nc()
    if trace:
        _ensure_ntff_hook()
        r = run_bass_kernel_spmd(nc, in_maps, list(range(N_CORES)), trace=True)
    else:
        # Never take the trace path implicitly (BASS_TRACE in env would pull
        # in profiling hooks that may not exist in the grading environment).
        prev = os.environ.get("BASS_NEVER_TRACE")
        os.environ["BASS_NEVER_TRACE"] = "1"
        try:
            r = run_bass_kernel_spmd(nc, in_maps, list(range(N_CORES)), trace=False)
        finally:
            if prev is None:
                os.environ.pop("BASS_NEVER_TRACE", None)
            else:
                os.environ["BASS_NEVER_TRACE"] = prev
    y = np.concatenate([r.results[c]["yT"].T for c in range(N_CORES)], axis=0)
    return y.reshape(x.shape).astype(np.float32), r.exec_time_ns


def kernel(**inputs):
    y, _ = run(inputs["x"], inputs["kernel"], inputs["bias"])
    return y
